# revision 35
# baseline (speedup 1.0000x reference)
"""Trainium2 Bass kernel for nn_Lookback: causal running-mean over T.

out[b, t, c] = (1/(t+1)) * sum_{s<=t} x[b, s, c],  x: [8, 4096, 1024] fp32.

Sharding: data-parallel over batch B — core b handles x[b] ([4096, 1024]).

Per-core algorithm (T tiled into 32 blocks of P=128 rows, processed as 8
QUADS; partition-REVERSED outputs: ps[p] holds global row 128k + 127 - p,
so each tile's running total lands on partition 0):
  For quad q (tiles t0..t3):
    ps_t0 = flipT @ x_t0                      (+ ones1 @ carryQ[q-1])
    ps_t1 = flipT @ x_t1 + ones128 @ x_t0     (+ ones1 @ carryQ[q-1])
    rb1 = ps_t1[0]        (partition 0 = running total through t1)
    ps_t2 = flipT @ x_t2 + ones1 @ rb1
    rb2 = ps_t2[0]
    ps_t3 = flipT @ x_t3 + ones1 @ rb2
    carryQ[q] = ps_t3[0]
    out_* = ps_* * 1/(t+1)   (per-partition scale at eviction, bf16)
  Chaining later tiles off the previous tile's PSUM row 0 (instead of a
  pair-sum matmul per pair) cuts PE work to 2304 cyc/tile — the PE clock
  is duty-cycle throttled to ~50%, so PE cycles dominate the runtime.
  All matmuls run in bf16 (1 cyc/row; fp32r measures 2-3 cyc/row here).
  x is cast f32->bf16 INSIDE the load DMA (SWDGE cast, full speed) —
  no staging ring or on-chip casts; the resident buffer is written once
  so all loads stream unthrottled from the start.
  The host un-reverses each 128-row block during the gather (numpy view).
  Output is stored as bf16 (tolerance 2e-2 >> bf16's ~2e-3), halving
  store traffic: 16 MiB loads + 8 MiB stores.

Engine split: GPSIMD issues cast-loads (queue 0), SP issues consts +
stores (queue 1), DVE: carry extracts + t2-evict, ACT: t0/t1/t3 evicts,
PE: matmuls only.
"""

import sys

import numpy as np

sys.path.insert(0, "/opt/trn_rl_repo")

import concourse.bass as bass
import concourse.mybir as mybir
import concourse.tile as tile
from concourse import bacc
from concourse.bass_utils import run_bass_kernel_spmd

B, T, C = 8, 4096, 1024
P = 128
NT = T // P          # 32 row tiles per core
NP = NT // 2         # 16 pairs (load/cast/store granularity)
NQ = NT // 4         # 8 quads (carry granularity)
CH = 512             # PSUM bank chunk (fp32)
NCH = C // CH
F32 = mybir.dt.float32
BF16 = mybir.dt.bfloat16

_cache = {}


def _consts():
    """Host-precomputed weight matrices (shared by all cores)."""
    # flipT[q, p] = [q <= 127 - p]: out partition p = global row 128k+127-p
    flip_t = np.triu(np.ones((P, P), np.float32))[:, ::-1].copy()
    ones1 = np.ones((1, P), np.float32)
    ones128 = np.ones((P, P), np.float32)
    # recip[p, k] = 1 / (128*k + 127 - p + 1)
    pidx = np.arange(P, dtype=np.float64)[:, None]      # [P, 1]
    kidx = np.arange(NT, dtype=np.float64)[None, :]     # [1, NT]
    recip = (1.0 / (128.0 * kidx + 128.0 - pidx)).astype(np.float32)
    import ml_dtypes
    bf = lambda a: a.astype(ml_dtypes.bfloat16)
    return bf(flip_t), bf(ones1), bf(ones128), recip


def _build():
    nc = bacc.Bacc("TRN2", target_bir_lowering=False, debug=False, num_devices=B)
    x_d = nc.dram_tensor("x", [T, C], F32, kind="ExternalInput").ap()
    flip_d = nc.dram_tensor("flip_t", [P, P], BF16, kind="ExternalInput").ap()
    ones1_d = nc.dram_tensor("ones1", [1, P], BF16, kind="ExternalInput").ap()
    ones128_d = nc.dram_tensor("ones128", [P, P], BF16, kind="ExternalInput").ap()
    r_d = nc.dram_tensor("recip", [P, NT], F32, kind="ExternalInput").ap()
    out_d = nc.dram_tensor("out", [T, C], BF16, kind="ExternalOutput").ap()

    x_pn = x_d.rearrange("(n p) c -> p n c", p=P)                # [P, NT, C]
    out_g = out_d.rearrange("(m n p) c -> m p n c", p=P, n=2)    # [16, P, 2, C]

    with tile.TileContext(nc) as tc:
        with (
            tc.tile_pool(name="const", bufs=1) as cp,
            tc.tile_pool(name="stg", bufs=4) as sp,
            tc.tile_pool(name="xbf", bufs=1) as xp,
            tc.tile_pool(name="carry", bufs=1) as kp,
            tc.tile_pool(name="ev", bufs=4) as ep,
            tc.tile_pool(name="ps", bufs=4, space=bass.MemorySpace.PSUM) as psp,
        ):
            flip_s = cp.tile([P, P], BF16)
            ones1_s = cp.tile([1, P], BF16)
            ones128_s = cp.tile([P, P], BF16)
            r_s = cp.tile([P, NT], F32)
            nc.sync.dma_start(flip_s[:], flip_d)
            nc.sync.dma_start(ones1_s[:], ones1_d)
            nc.sync.dma_start(ones128_s[:], ones128_d)
            nc.sync.dma_start(r_s[:], r_d)

            xr = xp.tile([P, NT, C], BF16)            # bf16 resident input
            # row buffers on partition 0: slots 0/1 quad carry, 2/3 rb1,
            # 4/5 rb2 (alternating per quad to break WAR chains)
            carry = kp.tile([1, 6, C], BF16)

            # SWDGE cast-loads: f32 DRAM -> bf16 SBUF directly in the DMA
            # (queue 0), no staging ring or on-chip casts; the resident
            # buffer is written once so all loads stream unthrottled
            for g in range(NP):
                nc.gpsimd.dma_start(xr[:, 2 * g:2 * g + 2, :],
                                    x_pn[:, 2 * g:2 * g + 2, :])

            # PE warm-up while the first loads land (short: the first real
            # matmul is otherwise warmup-gated, not load-gated)
            wu = psp.tile([P, C], F32, tag="ps")
            for _ in range(6):
                nc.tensor.matmul(wu[:, 0:P], flip_s[:], flip_s[:],
                                 start=True, stop=True)

            def mm(ps, w, xin, start, stop):
                for h in range(NCH):
                    sl = slice(h * CH, (h + 1) * CH)
                    nc.tensor.matmul(ps[:, sl], w, xin[:, sl],
                                     start=start, stop=stop)

            for q in range(NQ):
                t0, t1, t2, t3 = 4 * q, 4 * q + 1, 4 * q + 2, 4 * q + 3
                ps0 = psp.tile([P, C], F32, tag="ps")
                ps1 = psp.tile([P, C], F32, tag="ps")
                ps2 = psp.tile([P, C], F32, tag="ps")
                ps3 = psp.tile([P, C], F32, tag="ps")
                qs, rs1, rs2 = q % 2, 2 + q % 2, 4 + q % 2
                # carry-independent matmuls first
                mm(ps0, flip_s[:], xr[:, t0, :], True, q == 0)
                mm(ps1, flip_s[:], xr[:, t1, :], True, False)
                mm(ps1, ones128_s[:], xr[:, t0, :], False, q == 0)
                mm(ps2, flip_s[:], xr[:, t2, :], True, False)
                mm(ps3, flip_s[:], xr[:, t3, :], True, False)
                if q > 0:
                    # quad-carry broadcasts: ps1 first (the chain hangs off
                    # it), ps0 fills the extract window
                    mm(ps1, ones1_s[:], carry[:, (q - 1) % 2, :], False, True)
                    mm(ps0, ones1_s[:], carry[:, (q - 1) % 2, :], False, True)
                # intra-quad chain off PSUM row 0
                nc.vector.tensor_copy(carry[:, rs1, :], ps1[0:1, :])
                mm(ps2, ones1_s[:], carry[:, rs1, :], False, True)
                nc.vector.tensor_copy(carry[:, rs2, :], ps2[0:1, :])
                mm(ps3, ones1_s[:], carry[:, rs2, :], False, True)
                if q < NQ - 1:
                    nc.vector.tensor_copy(carry[:, qs, :], ps3[0:1, :])
                # evictions: t2 on DVE, rest on ACT; stores per pair
                oA = ep.tile([P, 2, C], BF16, tag="o")
                nc.scalar.activation(
                    oA[:, 1, :], ps1[:], mybir.ActivationFunctionType.Copy,
                    scale=r_s[:, t1:t1 + 1],
                )
                nc.scalar.activation(
                    oA[:, 0, :], ps0[:], mybir.ActivationFunctionType.Copy,
                    scale=r_s[:, t0:t0 + 1],
                )
                nc.sync.dma_start(out_g[2 * q], oA[:])
                oB = ep.tile([P, 2, C], BF16, tag="o")
                nc.vector.tensor_scalar_mul(oB[:, 0, :], ps2[:],
                                            r_s[:, t2:t2 + 1])
                if q == NQ - 1:
                    # tail: store the t2 half early, and split t3's
                    # eviction across ACT+DVE with its own small store
                    nc.sync.dma_start(out_g[2 * q + 1][:, 0, :], oB[:, 0, :])
                    nc.scalar.activation(
                        oB[:, 1, 0:CH], ps3[:, 0:CH],
                        mybir.ActivationFunctionType.Copy,
                        scale=r_s[:, t3:t3 + 1],
                    )
                    nc.vector.tensor_scalar_mul(oB[:, 1, CH:C], ps3[:, CH:C],
                                                r_s[:, t3:t3 + 1])
                    nc.sync.dma_start(out_g[2 * q + 1][:, 1, :], oB[:, 1, :])
                else:
                    nc.scalar.activation(
                        oB[:, 1, :], ps3[:], mybir.ActivationFunctionType.Copy,
                        scale=r_s[:, t3:t3 + 1],
                    )
                    nc.sync.dma_start(out_g[2 * q + 1], oB[:])

    nc.compile()
    return nc


def _run(x, trace=False):
    x = np.ascontiguousarray(x, dtype=np.float32)
    assert x.shape == (B, T, C)
    if "nc" not in _cache:
        _cache["nc"] = _build()
        _cache["consts"] = _consts()
    nc = _cache["nc"]
    flip_t, ones1, ones128, recip = _cache["consts"]
    in_maps = [
        {"x": x[b], "flip_t": flip_t, "ones1": ones1, "ones128": ones128,
         "recip": recip}
        for b in range(B)
    ]
    res = run_bass_kernel_spmd(nc, in_maps, core_ids=list(range(B)), trace=trace)
    # un-reverse each 128-row block (device wrote them partition-flipped)
    out = np.stack([
        np.asarray(res.results[b]["out"])
        .reshape(NT, P, C)[:, ::-1, :]
        .reshape(T, C)
        .astype(np.float32)
        for b in range(B)
    ])
    return out, res


def kernel(x):
    out, _ = _run(x, trace=False)
    return out
